# revision 19
# baseline (speedup 1.0000x reference)
"""Chamfer-distance (CDLoss) Trainium2 Bass kernel — single-pass, batched.

Problem: srcs, tgts [B=8, D=3, N=4096] fp32.
  P[b,i,j] = |s_i|^2 + |t_j|^2 - 2 s_i.t_j
  out = min(P, axis=1).mean() + min(P, axis=2).mean()   (scalar fp32)

Strategy (data-parallel over B across 8 NeuronCores, one batch per core):
  Each 128x4096 distance tile is computed ONCE (TensorE, bf16 hi/lo-split
  features, K=18) and consumed for BOTH reductions.  Column halves
  (2048) are the unit: TensorE -> PSUM f32 -> ScalarE cast to fp16 into
  a "ring" (4 tiles per ring, 2 rings per 8-unit block).

  All VectorE min work runs in fp16 2x_1P mode (measured: TT = 148 +
  w/2 cyc; tensor_reduce is always 1x = 141 + w cyc; scans/TTR/Max8 are
  slow or crash), and is BATCHED across 8 units with 3D APs to amortize
  instruction overhead:
    ROW (per-source mins): fold 2048->1024 per 4 units, then batched
      folds 1024->...->128 across 8 units, one batched 1x reduce
      [128,8,128] -> Rm[:, 8].
    COL (per-target mins): pair-tree across the 8 cast tiles
      (8->4->2->1) then one accumulate into A[128, 2048].
  Half finale: XBAR DMA-transpose A (DMA engines), fp16 fold + small
  reduce gives the 2048 column mins; overlaps the other half's compute.

  Per-core outputs: out_r [128, 64] f32 (row mins; col m+32*h, row p =
  source m*128+p) and out_c [128, 32] f16 (col mins, order-free);
  host does the means.
"""

import numpy as np
import ml_dtypes

_BF16 = ml_dtypes.bfloat16

# Problem geometry (hardcoded per contest contract).
_B = 8
_D = 3
_N = 4096
_P = 128              # partitions / queries per M-tile
_K = 18               # feature rows (see _features)
_NCORES = 8
_CHUNK = 2048         # PSUM chunk columns (4 banks) == column half

_prog_cache = {}

# test-harness knobs (the grading harness just calls kernel() and never
# touches these; default is the fast no-trace path)
TRACE = False
TRACE_CORES = [0]
LAST_RESULTS = None


def _build_program(n_pts=_N):
    import concourse.mybir as mybir
    import concourse.tile as tile
    from concourse import bacc

    P = _P
    MT = n_pts // P              # 32 m-tiles of 128 sources
    K = _K
    NH = n_pts // _CHUNK         # 2 column halves
    NB = _CHUNK // P             # 16 transpose blocks per half
    NBLK = MT // 8               # 4 eight-unit blocks per half
    f32 = mybir.dt.float32
    f16 = mybir.dt.float16
    bf16 = mybir.dt.bfloat16
    MIN = mybir.AluOpType.min
    AX = mybir.AxisListType.X

    nc = bacc.Bacc("TRN2", target_bir_lowering=False, debug=False,
                   num_devices=_NCORES)

    dram_w = nc.dram_tensor("w", [K, n_pts], bf16, kind="ExternalInput")
    dram_r = nc.dram_tensor("r", [K, n_pts], bf16, kind="ExternalInput")
    dram_out_r = nc.dram_tensor("out_r", [P, NH * MT], f32,
                                kind="ExternalOutput")
    dram_out_c = nc.dram_tensor("out_c", [P, NH * NB], f16,
                                kind="ExternalOutput")

    with tile.TileContext(nc) as tc:
        with (
            tc.tile_pool(name="const", bufs=1) as cpool,
            tc.tile_pool(name="ring", bufs=2) as ringpool,
            tc.tile_pool(name="stage", bufs=1) as spool,
            tc.tile_pool(name="acc", bufs=2) as apool,
            tc.tile_pool(name="trans", bufs=2) as tpool,
            tc.tile_pool(name="outs", bufs=2) as rpool,
            tc.tile_pool(name="psum", bufs=2, space="PSUM") as ppool,
        ):
            # Prologue: feature loads replicated into 4 PE row groups so
            # consecutive matmuls rotate groups (LDWEIGHTS overlaps
            # MATMUL).  Two HWDGE queues; neither is the ScalarE queue
            # (ScalarE is ~fully busy casting).
            sbW = cpool.tile([128, n_pts], bf16, tag="sbW", name="sbW")
            sbR = cpool.tile([128, n_pts], bf16, tag="sbR", name="sbR")
            # All loads on the sync queue, W/R interleaved per group, so
            # group g's pair lands early and the ScalarE queue stays free
            # for the first cast (its queue is strict FIFO).  Two small
            # priority slices cover exactly unit 0's operands (m-tile 0
            # weights + first column half) so its matmuls start ~2us
            # earlier; the group-0 bulk loads exclude that range to keep
            # the dependency on the priority DMAs only.
            nc.sync.dma_start(sbW[0:K, 0:P], dram_w[:, 0:P])
            nc.sync.dma_start(sbR[0:K, 0:_CHUNK], dram_r[:, 0:_CHUNK])
            nc.sync.dma_start(sbW[0:K, P:], dram_w[:, P:])
            nc.sync.dma_start(sbR[0:K, _CHUNK:], dram_r[:, _CHUNK:])
            for g in range(1, 4):
                nc.sync.dma_start(sbW[32 * g:32 * g + K, :], dram_w[:])
                nc.sync.dma_start(sbR[32 * g:32 * g + K, :], dram_r[:])

            V = nc.vector
            for h in range(NH):
                A = apool.tile([P, _CHUNK], f16, tag="A", name=f"A{h}")
                # cross-block staging: per-block COL results and ROW
                # fold-to-256 partials, combined once per half
                CC = spool.tile([P, NBLK, _CHUNK], f16, tag="CC")
                SS = spool.tile([P, NBLK, 8, 256], f16, tag="SS")
                for blk in range(NBLK):
                    rings = [
                        ringpool.tile([P, 4, _CHUNK], f16, tag=f"ring{i}",
                                      name=f"ring{i}")
                        for i in range(2)
                    ]
                    S1 = spool.tile([P, 8, 1024], f16, tag="S1")
                    C2 = spool.tile([P, 4, _CHUNK], f16, tag="C2")
                    first = h == 0 and blk == 0
                    for j in range(8):
                        m = blk * 8 + j
                        ring, jj = rings[j // 4], j % 4
                        ps = ppool.tile([P, _CHUNK], f32, tag="ps")
                        for q in range(4):
                            # row group rotates per 512-col matmul so
                            # LDWEIGHTS overlaps in-flight MATMULs.  The
                            # first 4 units pin all their matmuls to one
                            # group so unit u depends only on the u-th
                            # prologue DMA pair (shorter kernel-start
                            # ramp); PE has large slack there.
                            g = j if first and j < 4 else q
                            col = _CHUNK * h + 512 * q
                            nc.tensor.matmul(
                                ps[:, 512 * q:512 * (q + 1)],
                                sbW[32 * g:32 * g + K, m * P:(m + 1) * P],
                                sbR[32 * g:32 * g + K, col:col + 512],
                                start=True, stop=True,
                                tile_position=(32 * g, 0),
                            )
                        nc.scalar.copy(ring[:, jj, :], ps[:])
                        if first and j % 2 == 1:
                            # kernel-start ramp: sub-batch by 2 so VectorE
                            # starts two casts earlier
                            half, sub = j // 4, (j // 2) % 2
                            V.tensor_tensor(
                                S1[:, j - 1:j + 1, :],
                                ring[:, 2 * sub:2 * sub + 2, 0:1024],
                                ring[:, 2 * sub:2 * sub + 2, 1024:2048],
                                op=MIN)
                            V.tensor_tensor(
                                C2[:, j // 2:j // 2 + 1, :],
                                ring[:, 2 * sub:2 * sub + 1, :],
                                ring[:, 2 * sub + 1:2 * sub + 2, :],
                                op=MIN)
                        elif not first and (j == 3 or j == 7):
                            half = j // 4
                            # ROW fold level 1 for these 4 units
                            V.tensor_tensor(
                                S1[:, 4 * half:4 * half + 4, :],
                                ring[:, :, 0:1024], ring[:, :, 1024:2048],
                                op=MIN)
                            # COL pair tree level 1: (0,1),(2,3)
                            V.tensor_tensor(
                                C2[:, 2 * half:2 * half + 2, :],
                                ring[:, 0:4:2, :], ring[:, 1:4:2, :],
                                op=MIN)
                    # block tail: COL tree merge into CC, ROW fold into SS
                    C4 = spool.tile([P, 2, _CHUNK], f16, tag="C4")
                    V.tensor_tensor(C4[:], C2[:, 0:4:2, :],
                                    C2[:, 1:4:2, :], op=MIN)
                    V.tensor_tensor(CC[:, blk, :], C4[:, 0, :],
                                    C4[:, 1, :], op=MIN)
                    S2 = spool.tile([P, 8, 512], f16, tag="S2")
                    V.tensor_tensor(S2[:], S1[:, :, 0:512],
                                    S1[:, :, 512:1024], op=MIN)
                    V.tensor_tensor(SS[:, blk, :, :], S2[:, :, 0:256],
                                    S2[:, :, 256:512], op=MIN)
                # Half finale.  COL: combine per-block results, then
                # XBAR-transpose A so column mins become a free-axis
                # fold+reduce (DMA engines move it while VectorE runs
                # the ROW tail).
                D1 = spool.tile([P, 2, _CHUNK], f16, tag="D1")
                V.tensor_tensor(D1[:], CC[:, 0:2, :], CC[:, 2:4, :],
                                op=MIN)
                V.tensor_tensor(A[:], D1[:, 0, :], D1[:, 1, :], op=MIN)
                TA = tpool.tile([P, NB, P], f16, tag="TA", name=f"TA{h}")
                nc.sync.dma_start_transpose(TA[:], A[:])
                T4 = spool.tile([P, NBLK, 8, 128], f16, tag="T4")
                V.tensor_tensor(T4[:], SS[:, :, :, 0:128],
                                SS[:, :, :, 128:256], op=MIN)
                T5 = spool.tile([P, NBLK, 8, 64], f16, tag="T5")
                V.tensor_tensor(T5[:], T4[:, :, :, 0:64],
                                T4[:, :, :, 64:128], op=MIN)
                T6 = spool.tile([P, NBLK, 8, 32], f16, tag="T6")
                V.tensor_tensor(T6[:], T5[:, :, :, 0:32],
                                T5[:, :, :, 32:64], op=MIN)
                Rm = rpool.tile([P, MT], f32, tag="Rm", name=f"Rm{h}")
                V.tensor_reduce(Rm[:], T6[:], axis=AX, op=MIN)
                F1 = spool.tile([P, NB, 64], f16, tag="F1")
                V.tensor_tensor(F1[:], TA[:, :, 0:64], TA[:, :, 64:128],
                                op=MIN)
                F2 = spool.tile([P, NB, 32], f16, tag="F2")
                V.tensor_tensor(F2[:], F1[:, :, 0:32], F1[:, :, 32:64],
                                op=MIN)
                C = rpool.tile([P, NB], f16, tag="C", name=f"C{h}")
                V.tensor_reduce(C[:], F2[:], axis=AX, op=MIN)
                nc.sync.dma_start(
                    dram_out_r[:, h * MT:(h + 1) * MT], Rm[:])
                nc.scalar.dma_start(
                    dram_out_c[:, h * NB:(h + 1) * NB], C[:])

    nc.compile()
    return nc


def _get_program(n_pts=_N):
    if n_pts not in _prog_cache:
        _prog_cache[n_pts] = _build_program(n_pts)
    return _prog_cache[n_pts]


def _split_bf16(x32):
    """x32 fp32 -> (hi, lo) bf16 with hi+lo ~= x to ~2^-18 rel."""
    hi = x32.astype(_BF16)
    lo = (x32 - hi.astype(np.float32)).astype(_BF16)
    return hi, lo


def _split3(x64):
    """fp64 vector -> 3 bf16 terms summing to x to ~2^-27 rel."""
    t0 = x64.astype(_BF16)
    r = x64 - t0.astype(np.float64)
    t1 = r.astype(_BF16)
    r2 = r - t1.astype(np.float64)
    t2 = r2.astype(_BF16)
    return t0, t1, t2


def _features(q, c, n_pts):
    """Feature tensors for the distance matmul.

    q: query points  [3, N] fp32; c: candidate points [3, N] fp32.
    Returns (W [18, N] bf16, R [18, N] bf16) with
      (W.T @ R)[i, j] ~= |q~_i - c~_j|^2
    with ~ the bf16-split (hi+lo) values, exact to ~2e-6.
    """
    q_hi, q_lo = _split_bf16(q)
    c_hi, c_lo = _split_bf16(c)
    q_t = q_hi.astype(np.float32) + q_lo.astype(np.float32)
    c_t = c_hi.astype(np.float32) + c_lo.astype(np.float32)

    U = (c_t.astype(np.float64) ** 2).sum(axis=0)   # candidate norms
    u0, u1, u2 = _split3(U)
    V = (q_t.astype(np.float64) ** 2).sum(axis=0)   # query norms
    v0, v1, v2 = _split3(V)

    m2q_hi = (-2.0 * q_hi.astype(np.float32)).astype(_BF16)
    m2q_lo = (-2.0 * q_lo.astype(np.float32)).astype(_BF16)
    ones = np.ones(n_pts, dtype=_BF16)

    Wg = np.concatenate([
        m2q_hi, m2q_hi, m2q_lo, m2q_lo,
        np.stack([ones, ones, ones]),
        np.stack([v0, v1, v2]),
    ], axis=0).astype(_BF16)              # [18, N]
    Rg = np.concatenate([
        c_hi, c_lo, c_hi, c_lo,
        np.stack([u0, u1, u2]),
        np.stack([ones, ones, ones]),
    ], axis=0).astype(_BF16)              # [18, N]

    return Wg, Rg


def kernel(srcs, tgts):
    import concourse.bass_utils as bass_utils

    srcs = np.asarray(srcs, dtype=np.float32)
    tgts = np.asarray(tgts, dtype=np.float32)
    B = srcs.shape[0]
    assert srcs.shape == (B, _D, _N) and tgts.shape == (B, _D, _N)

    nc = _get_program()

    in_maps = []
    for b in range(B):
        W, R = _features(srcs[b], tgts[b], _N)  # queries = sources
        in_maps.append({"w": W, "r": R})

    res = None
    for attempt in range(3):
        try:
            res = bass_utils.run_bass_kernel_spmd(
                nc, in_maps, core_ids=list(range(_NCORES)),
                trace=TRACE, trace_cores=TRACE_CORES if TRACE else None,
            )
            break
        except Exception:
            # transient NRT/device hiccups have been observed; retry
            if attempt == 2:
                raise
            import time
            time.sleep(3.0)
    global LAST_RESULTS
    LAST_RESULTS = res

    total = 0.0
    for b in range(B):
        out_r = res.results[b]["out_r"]   # [128, 64] f32 per-source mins
        out_c = res.results[b]["out_c"]   # [128, 32] f16 per-target mins
        row = np.minimum(out_r[:, :32], out_r[:, 32:]).astype(np.float64)
        col = out_c.astype(np.float64)
        # reference: min(P, axis=1).mean() -> per-target mins (col);
        #            min(P, axis=2).mean() -> per-source mins (row)
        total += col.mean() + row.mean()

    return np.float32(total / B)


# revision 20
# speedup vs baseline: 1.0099x; 1.0099x over previous
"""Chamfer-distance (CDLoss) Trainium2 Bass kernel — single-pass, batched.

Problem: srcs, tgts [B=8, D=3, N=4096] fp32.
  P[b,i,j] = |s_i|^2 + |t_j|^2 - 2 s_i.t_j
  out = min(P, axis=1).mean() + min(P, axis=2).mean()   (scalar fp32)

Strategy (data-parallel over B across 8 NeuronCores, one batch per core):
  Each 128x4096 distance tile is computed ONCE (TensorE, bf16 hi/lo-split
  features, K=18) and consumed for BOTH reductions.  Column halves
  (2048) are the unit: TensorE -> PSUM f32 -> ScalarE cast to fp16 into
  a "ring" (4 tiles per ring, 2 rings per 8-unit block).

  All VectorE min work runs in fp16 2x_1P mode (measured: TT = 148 +
  w/2 cyc; tensor_reduce is always 1x = 141 + w cyc; scans/TTR/Max8 are
  slow or crash), and is BATCHED across 8 units with 3D APs to amortize
  instruction overhead:
    ROW (per-source mins): fold 2048->1024 per 4 units, then batched
      folds 1024->...->128 across 8 units, one batched 1x reduce
      [128,8,128] -> Rm[:, 8].
    COL (per-target mins): pair-tree across the 8 cast tiles
      (8->4->2->1) then one accumulate into A[128, 2048].
  Half finale: XBAR DMA-transpose A (DMA engines), fp16 fold + small
  reduce gives the 2048 column mins; overlaps the other half's compute.

  Per-core outputs: out_r [128, 64] f32 (row mins; col m+32*h, row p =
  source m*128+p) and out_c [128, 32] f16 (col mins, order-free);
  host does the means.
"""

import numpy as np
import ml_dtypes

_BF16 = ml_dtypes.bfloat16

# Problem geometry (hardcoded per contest contract).
_B = 8
_D = 3
_N = 4096
_P = 128              # partitions / queries per M-tile
_K = 18               # feature rows (see _features)
_NCORES = 8
_CHUNK = 2048         # PSUM chunk columns (4 banks) == column half

_prog_cache = {}

# test-harness knobs (the grading harness just calls kernel() and never
# touches these; default is the fast no-trace path)
TRACE = False
TRACE_CORES = [0]
LAST_RESULTS = None


def _build_program(n_pts=_N):
    import concourse.mybir as mybir
    import concourse.tile as tile
    from concourse import bacc

    P = _P
    MT = n_pts // P              # 32 m-tiles of 128 sources
    K = _K
    NH = n_pts // _CHUNK         # 2 column halves
    NB = _CHUNK // P             # 16 transpose blocks per half
    NBLK = MT // 8               # 4 eight-unit blocks per half
    f32 = mybir.dt.float32
    f16 = mybir.dt.float16
    bf16 = mybir.dt.bfloat16
    MIN = mybir.AluOpType.min
    AX = mybir.AxisListType.X

    nc = bacc.Bacc("TRN2", target_bir_lowering=False, debug=False,
                   num_devices=_NCORES)

    dram_w = nc.dram_tensor("w", [K, n_pts], bf16, kind="ExternalInput")
    dram_r = nc.dram_tensor("r", [K, n_pts], bf16, kind="ExternalInput")
    dram_out_r = nc.dram_tensor("out_r", [P, NH * MT], f32,
                                kind="ExternalOutput")
    dram_out_c = nc.dram_tensor("out_c", [P, NH * NB], f16,
                                kind="ExternalOutput")

    with tile.TileContext(nc) as tc:
        with (
            tc.tile_pool(name="const", bufs=1) as cpool,
            tc.tile_pool(name="ring", bufs=2) as ringpool,
            tc.tile_pool(name="stage", bufs=1) as spool,
            tc.tile_pool(name="acc", bufs=2) as apool,
            tc.tile_pool(name="trans", bufs=2) as tpool,
            tc.tile_pool(name="outs", bufs=2) as rpool,
            tc.tile_pool(name="psum", bufs=2, space="PSUM") as ppool,
        ):
            # Prologue: feature loads replicated into 4 PE row groups so
            # consecutive matmuls rotate groups (LDWEIGHTS overlaps
            # MATMUL).  Two HWDGE queues; neither is the ScalarE queue
            # (ScalarE is ~fully busy casting).
            sbW = cpool.tile([128, n_pts], bf16, tag="sbW", name="sbW")
            sbR = cpool.tile([128, n_pts], bf16, tag="sbR", name="sbR")
            # All loads on the sync queue, W/R interleaved per group, so
            # group g's pair lands early and the ScalarE queue stays free
            # for the first cast (its queue is strict FIFO).  Two small
            # priority slices cover exactly unit 0's operands (m-tile 0
            # weights + first column half) so its matmuls start ~2us
            # earlier; the group-0 bulk loads exclude that range to keep
            # the dependency on the priority DMAs only.
            nc.sync.dma_start(sbW[0:K, 0:P], dram_w[:, 0:P])
            nc.sync.dma_start(sbR[0:K, 0:_CHUNK], dram_r[:, 0:_CHUNK])
            for g in range(1, 4):
                nc.sync.dma_start(sbW[32 * g:32 * g + K, :], dram_w[:])
                nc.sync.dma_start(sbR[32 * g:32 * g + K, :], dram_r[:])
            # group-0 bulk (first needed by unit 4) goes last
            nc.sync.dma_start(sbW[0:K, P:], dram_w[:, P:])
            nc.sync.dma_start(sbR[0:K, _CHUNK:], dram_r[:, _CHUNK:])

            V = nc.vector
            for h in range(NH):
                A = apool.tile([P, _CHUNK], f16, tag="A", name=f"A{h}")
                # cross-block staging: per-block COL results and ROW
                # fold-to-256 partials, combined once per half
                CC = spool.tile([P, NBLK, _CHUNK], f16, tag="CC")
                SS = spool.tile([P, NBLK, 8, 256], f16, tag="SS")
                for blk in range(NBLK):
                    rings = [
                        ringpool.tile([P, 4, _CHUNK], f16, tag=f"ring{i}",
                                      name=f"ring{i}")
                        for i in range(2)
                    ]
                    S1 = spool.tile([P, 8, 1024], f16, tag="S1")
                    C2 = spool.tile([P, 4, _CHUNK], f16, tag="C2")
                    first = h == 0 and blk == 0
                    for j in range(8):
                        m = blk * 8 + j
                        ring, jj = rings[j // 4], j % 4
                        ps = ppool.tile([P, _CHUNK], f32, tag="ps")
                        for q in range(4):
                            # row group rotates per 512-col matmul so
                            # LDWEIGHTS overlaps in-flight MATMULs.  The
                            # first 4 units pin all their matmuls to one
                            # group so unit u depends only on the u-th
                            # prologue DMA pair (shorter kernel-start
                            # ramp); PE has large slack there.
                            g = j if first and j < 4 else q
                            col = _CHUNK * h + 512 * q
                            nc.tensor.matmul(
                                ps[:, 512 * q:512 * (q + 1)],
                                sbW[32 * g:32 * g + K, m * P:(m + 1) * P],
                                sbR[32 * g:32 * g + K, col:col + 512],
                                start=True, stop=True,
                                tile_position=(32 * g, 0),
                            )
                        nc.scalar.copy(ring[:, jj, :], ps[:])
                        if first and j % 2 == 1:
                            # kernel-start ramp: sub-batch by 2 so VectorE
                            # starts two casts earlier
                            half, sub = j // 4, (j // 2) % 2
                            V.tensor_tensor(
                                S1[:, j - 1:j + 1, :],
                                ring[:, 2 * sub:2 * sub + 2, 0:1024],
                                ring[:, 2 * sub:2 * sub + 2, 1024:2048],
                                op=MIN)
                            V.tensor_tensor(
                                C2[:, j // 2:j // 2 + 1, :],
                                ring[:, 2 * sub:2 * sub + 1, :],
                                ring[:, 2 * sub + 1:2 * sub + 2, :],
                                op=MIN)
                        elif not first and (j == 3 or j == 7):
                            half = j // 4
                            # ROW fold level 1 for these 4 units
                            V.tensor_tensor(
                                S1[:, 4 * half:4 * half + 4, :],
                                ring[:, :, 0:1024], ring[:, :, 1024:2048],
                                op=MIN)
                            # COL pair tree level 1: (0,1),(2,3)
                            V.tensor_tensor(
                                C2[:, 2 * half:2 * half + 2, :],
                                ring[:, 0:4:2, :], ring[:, 1:4:2, :],
                                op=MIN)
                    # block tail: COL tree merge into CC, ROW fold into SS
                    C4 = spool.tile([P, 2, _CHUNK], f16, tag="C4")
                    V.tensor_tensor(C4[:], C2[:, 0:4:2, :],
                                    C2[:, 1:4:2, :], op=MIN)
                    V.tensor_tensor(CC[:, blk, :], C4[:, 0, :],
                                    C4[:, 1, :], op=MIN)
                    S2 = spool.tile([P, 8, 512], f16, tag="S2")
                    V.tensor_tensor(S2[:], S1[:, :, 0:512],
                                    S1[:, :, 512:1024], op=MIN)
                    V.tensor_tensor(SS[:, blk, :, :], S2[:, :, 0:256],
                                    S2[:, :, 256:512], op=MIN)
                # Half finale.  COL: combine per-block results, then
                # XBAR-transpose A so column mins become a free-axis
                # fold+reduce (DMA engines move it while VectorE runs
                # the ROW tail).
                D1 = spool.tile([P, 2, _CHUNK], f16, tag="D1")
                V.tensor_tensor(D1[:], CC[:, 0:2, :], CC[:, 2:4, :],
                                op=MIN)
                V.tensor_tensor(A[:], D1[:, 0, :], D1[:, 1, :], op=MIN)
                TA = tpool.tile([P, NB, P], f16, tag="TA", name=f"TA{h}")
                nc.sync.dma_start_transpose(TA[:], A[:])
                T4 = spool.tile([P, NBLK, 8, 128], f16, tag="T4")
                V.tensor_tensor(T4[:], SS[:, :, :, 0:128],
                                SS[:, :, :, 128:256], op=MIN)
                T5 = spool.tile([P, NBLK, 8, 64], f16, tag="T5")
                V.tensor_tensor(T5[:], T4[:, :, :, 0:64],
                                T4[:, :, :, 64:128], op=MIN)
                T6 = spool.tile([P, NBLK, 8, 32], f16, tag="T6")
                V.tensor_tensor(T6[:], T5[:, :, :, 0:32],
                                T5[:, :, :, 32:64], op=MIN)
                Rm = rpool.tile([P, MT], f32, tag="Rm", name=f"Rm{h}")
                V.tensor_reduce(Rm[:], T6[:], axis=AX, op=MIN)
                F1 = spool.tile([P, NB, 64], f16, tag="F1")
                V.tensor_tensor(F1[:], TA[:, :, 0:64], TA[:, :, 64:128],
                                op=MIN)
                F2 = spool.tile([P, NB, 32], f16, tag="F2")
                V.tensor_tensor(F2[:], F1[:, :, 0:32], F1[:, :, 32:64],
                                op=MIN)
                C = rpool.tile([P, NB], f16, tag="C", name=f"C{h}")
                V.tensor_reduce(C[:], F2[:], axis=AX, op=MIN)
                nc.sync.dma_start(
                    dram_out_r[:, h * MT:(h + 1) * MT], Rm[:])
                nc.scalar.dma_start(
                    dram_out_c[:, h * NB:(h + 1) * NB], C[:])

    nc.compile()
    return nc


def _get_program(n_pts=_N):
    if n_pts not in _prog_cache:
        _prog_cache[n_pts] = _build_program(n_pts)
    return _prog_cache[n_pts]


def _split_bf16(x32):
    """x32 fp32 -> (hi, lo) bf16 with hi+lo ~= x to ~2^-18 rel."""
    hi = x32.astype(_BF16)
    lo = (x32 - hi.astype(np.float32)).astype(_BF16)
    return hi, lo


def _split3(x64):
    """fp64 vector -> 3 bf16 terms summing to x to ~2^-27 rel."""
    t0 = x64.astype(_BF16)
    r = x64 - t0.astype(np.float64)
    t1 = r.astype(_BF16)
    r2 = r - t1.astype(np.float64)
    t2 = r2.astype(_BF16)
    return t0, t1, t2


def _features(q, c, n_pts):
    """Feature tensors for the distance matmul.

    q: query points  [3, N] fp32; c: candidate points [3, N] fp32.
    Returns (W [18, N] bf16, R [18, N] bf16) with
      (W.T @ R)[i, j] ~= |q~_i - c~_j|^2
    with ~ the bf16-split (hi+lo) values, exact to ~2e-6.
    """
    q_hi, q_lo = _split_bf16(q)
    c_hi, c_lo = _split_bf16(c)
    q_t = q_hi.astype(np.float32) + q_lo.astype(np.float32)
    c_t = c_hi.astype(np.float32) + c_lo.astype(np.float32)

    U = (c_t.astype(np.float64) ** 2).sum(axis=0)   # candidate norms
    u0, u1, u2 = _split3(U)
    V = (q_t.astype(np.float64) ** 2).sum(axis=0)   # query norms
    v0, v1, v2 = _split3(V)

    m2q_hi = (-2.0 * q_hi.astype(np.float32)).astype(_BF16)
    m2q_lo = (-2.0 * q_lo.astype(np.float32)).astype(_BF16)
    ones = np.ones(n_pts, dtype=_BF16)

    Wg = np.concatenate([
        m2q_hi, m2q_hi, m2q_lo, m2q_lo,
        np.stack([ones, ones, ones]),
        np.stack([v0, v1, v2]),
    ], axis=0).astype(_BF16)              # [18, N]
    Rg = np.concatenate([
        c_hi, c_lo, c_hi, c_lo,
        np.stack([u0, u1, u2]),
        np.stack([ones, ones, ones]),
    ], axis=0).astype(_BF16)              # [18, N]

    return Wg, Rg


def kernel(srcs, tgts):
    import concourse.bass_utils as bass_utils

    srcs = np.asarray(srcs, dtype=np.float32)
    tgts = np.asarray(tgts, dtype=np.float32)
    B = srcs.shape[0]
    assert srcs.shape == (B, _D, _N) and tgts.shape == (B, _D, _N)

    nc = _get_program()

    in_maps = []
    for b in range(B):
        W, R = _features(srcs[b], tgts[b], _N)  # queries = sources
        in_maps.append({"w": W, "r": R})

    res = None
    for attempt in range(3):
        try:
            res = bass_utils.run_bass_kernel_spmd(
                nc, in_maps, core_ids=list(range(_NCORES)),
                trace=TRACE, trace_cores=TRACE_CORES if TRACE else None,
            )
            break
        except Exception:
            # transient NRT/device hiccups have been observed; retry
            if attempt == 2:
                raise
            import time
            time.sleep(3.0)
    global LAST_RESULTS
    LAST_RESULTS = res

    total = 0.0
    for b in range(B):
        out_r = res.results[b]["out_r"]   # [128, 64] f32 per-source mins
        out_c = res.results[b]["out_c"]   # [128, 32] f16 per-target mins
        row = np.minimum(out_r[:, :32], out_r[:, 32:]).astype(np.float64)
        col = out_c.astype(np.float64)
        # reference: min(P, axis=1).mean() -> per-target mins (col);
        #            min(P, axis=2).mean() -> per-source mins (row)
        total += col.mean() + row.mean()

    return np.float32(total / B)


# revision 21
# speedup vs baseline: 1.0176x; 1.0076x over previous
"""Chamfer-distance (CDLoss) Trainium2 Bass kernel — single-pass, batched.

Problem: srcs, tgts [B=8, D=3, N=4096] fp32.
  P[b,i,j] = |s_i|^2 + |t_j|^2 - 2 s_i.t_j
  out = min(P, axis=1).mean() + min(P, axis=2).mean()   (scalar fp32)

Strategy (data-parallel over B across 8 NeuronCores, one batch per core):
  Each 128x4096 distance tile is computed ONCE (TensorE, bf16 hi/lo-split
  features, K=18) and consumed for BOTH reductions.  Column halves
  (2048) are the unit: TensorE -> PSUM f32 -> ScalarE cast to fp16 into
  a "ring" (4 tiles per ring, 2 rings per 8-unit block).

  All VectorE min work runs in fp16 2x_1P mode (measured: TT = 148 +
  w/2 cyc; tensor_reduce is always 1x = 141 + w cyc; scans/TTR/Max8 are
  slow or crash), and is BATCHED across 8 units with 3D APs to amortize
  instruction overhead:
    ROW (per-source mins): fold 2048->1024 per 4 units, then batched
      folds 1024->...->128 across 8 units, one batched 1x reduce
      [128,8,128] -> Rm[:, 8].
    COL (per-target mins): pair-tree across the 8 cast tiles
      (8->4->2->1) then one accumulate into A[128, 2048].
  Half finale: XBAR DMA-transpose A (DMA engines), fp16 fold + small
  reduce gives the 2048 column mins; overlaps the other half's compute.

  Per-core outputs: out_r [128, 64] f32 (row mins; col m+32*h, row p =
  source m*128+p) and out_c [128, 32] f16 (col mins, order-free);
  host does the means.
"""

import numpy as np
import ml_dtypes

_BF16 = ml_dtypes.bfloat16

# Problem geometry (hardcoded per contest contract).
_B = 8
_D = 3
_N = 4096
_P = 128              # partitions / queries per M-tile
_K = 18               # feature rows (see _features)
_NCORES = 8
_CHUNK = 2048         # PSUM chunk columns (4 banks) == column half

_prog_cache = {}

# test-harness knobs (the grading harness just calls kernel() and never
# touches these; default is the fast no-trace path)
TRACE = False
TRACE_CORES = [0]
LAST_RESULTS = None


def _build_program(n_pts=_N):
    import concourse.mybir as mybir
    import concourse.tile as tile
    from concourse import bacc

    P = _P
    MT = n_pts // P              # 32 m-tiles of 128 sources
    K = _K
    NH = n_pts // _CHUNK         # 2 column halves
    NB = _CHUNK // P             # 16 transpose blocks per half
    NBLK = MT // 8               # 4 eight-unit blocks per half
    f32 = mybir.dt.float32
    f16 = mybir.dt.float16
    bf16 = mybir.dt.bfloat16
    MIN = mybir.AluOpType.min
    AX = mybir.AxisListType.X

    nc = bacc.Bacc("TRN2", target_bir_lowering=False, debug=False,
                   num_devices=_NCORES)

    dram_w = nc.dram_tensor("w", [K, n_pts], bf16, kind="ExternalInput")
    dram_r = nc.dram_tensor("r", [K, n_pts], bf16, kind="ExternalInput")
    dram_out_r = nc.dram_tensor("out_r", [P, NH * MT], f32,
                                kind="ExternalOutput")
    dram_out_c = nc.dram_tensor("out_c", [P, NH * NB], f16,
                                kind="ExternalOutput")

    with tile.TileContext(nc) as tc:
        with (
            tc.tile_pool(name="const", bufs=1) as cpool,
            tc.tile_pool(name="ring", bufs=2) as ringpool,
            tc.tile_pool(name="stage", bufs=1) as spool,
            tc.tile_pool(name="acc", bufs=2) as apool,
            tc.tile_pool(name="trans", bufs=2) as tpool,
            tc.tile_pool(name="outs", bufs=2) as rpool,
            tc.tile_pool(name="psum", bufs=2, space="PSUM") as ppool,
        ):
            # Prologue: feature loads replicated into 4 PE row groups so
            # consecutive matmuls rotate groups (LDWEIGHTS overlaps
            # MATMUL).  Two HWDGE queues; neither is the ScalarE queue
            # (ScalarE is ~fully busy casting).
            sbW = cpool.tile([128, n_pts], bf16, tag="sbW", name="sbW")
            sbR = cpool.tile([128, n_pts], bf16, tag="sbR", name="sbR")
            # All loads on the sync queue, W/R interleaved per group, so
            # group g's pair lands early and the ScalarE queue stays free
            # for the first cast (its queue is strict FIFO).  Two small
            # priority slices cover exactly unit 0's operands (m-tile 0
            # weights + first column half) so its matmuls start ~2us
            # earlier; the group-0 bulk loads exclude that range to keep
            # the dependency on the priority DMAs only.
            nc.sync.dma_start(sbW[0:K, 0:P], dram_w[:, 0:P])
            nc.sync.dma_start(sbR[0:K, 0:_CHUNK], dram_r[:, 0:_CHUNK])
            # group 1 rides the otherwise-idle ScalarE queue in parallel
            # (it drains well before the first cast is ready to issue)
            nc.scalar.dma_start(sbW[32:32 + K, :], dram_w[:])
            nc.scalar.dma_start(sbR[32:32 + K, :], dram_r[:])
            for g in range(2, 4):
                nc.sync.dma_start(sbW[32 * g:32 * g + K, :], dram_w[:])
                nc.sync.dma_start(sbR[32 * g:32 * g + K, :], dram_r[:])
            # group-0 bulk (first needed by unit 4) goes last
            nc.sync.dma_start(sbW[0:K, P:], dram_w[:, P:])
            nc.sync.dma_start(sbR[0:K, _CHUNK:], dram_r[:, _CHUNK:])

            V = nc.vector
            for h in range(NH):
                A = apool.tile([P, _CHUNK], f16, tag="A", name=f"A{h}")
                # cross-block staging: per-block COL results and ROW
                # fold-to-256 partials, combined once per half
                CC = spool.tile([P, NBLK, _CHUNK], f16, tag="CC")
                SS = spool.tile([P, NBLK, 8, 256], f16, tag="SS")
                for blk in range(NBLK):
                    rings = [
                        ringpool.tile([P, 4, _CHUNK], f16, tag=f"ring{i}",
                                      name=f"ring{i}")
                        for i in range(2)
                    ]
                    S1 = spool.tile([P, 8, 1024], f16, tag="S1")
                    C2 = spool.tile([P, 4, _CHUNK], f16, tag="C2")
                    first = h == 0 and blk == 0
                    for j in range(8):
                        m = blk * 8 + j
                        ring, jj = rings[j // 4], j % 4
                        ps = ppool.tile([P, _CHUNK], f32, tag="ps")
                        for q in range(4):
                            # row group rotates per 512-col matmul so
                            # LDWEIGHTS overlaps in-flight MATMULs.  The
                            # first 4 units pin all their matmuls to one
                            # group so unit u depends only on the u-th
                            # prologue DMA pair (shorter kernel-start
                            # ramp); PE has large slack there.
                            g = j if first and j < 4 else q
                            col = _CHUNK * h + 512 * q
                            nc.tensor.matmul(
                                ps[:, 512 * q:512 * (q + 1)],
                                sbW[32 * g:32 * g + K, m * P:(m + 1) * P],
                                sbR[32 * g:32 * g + K, col:col + 512],
                                start=True, stop=True,
                                tile_position=(32 * g, 0),
                            )
                        nc.scalar.copy(ring[:, jj, :], ps[:])
                        if first and j % 2 == 1:
                            # kernel-start ramp: sub-batch by 2 so VectorE
                            # starts two casts earlier
                            half, sub = j // 4, (j // 2) % 2
                            V.tensor_tensor(
                                S1[:, j - 1:j + 1, :],
                                ring[:, 2 * sub:2 * sub + 2, 0:1024],
                                ring[:, 2 * sub:2 * sub + 2, 1024:2048],
                                op=MIN)
                            V.tensor_tensor(
                                C2[:, j // 2:j // 2 + 1, :],
                                ring[:, 2 * sub:2 * sub + 1, :],
                                ring[:, 2 * sub + 1:2 * sub + 2, :],
                                op=MIN)
                        elif not first and (j == 3 or j == 7):
                            half = j // 4
                            # ROW fold level 1 for these 4 units
                            V.tensor_tensor(
                                S1[:, 4 * half:4 * half + 4, :],
                                ring[:, :, 0:1024], ring[:, :, 1024:2048],
                                op=MIN)
                            # COL pair tree level 1: (0,1),(2,3)
                            V.tensor_tensor(
                                C2[:, 2 * half:2 * half + 2, :],
                                ring[:, 0:4:2, :], ring[:, 1:4:2, :],
                                op=MIN)
                    # block tail: COL tree merge into CC, ROW fold into SS
                    C4 = spool.tile([P, 2, _CHUNK], f16, tag="C4")
                    V.tensor_tensor(C4[:], C2[:, 0:4:2, :],
                                    C2[:, 1:4:2, :], op=MIN)
                    V.tensor_tensor(CC[:, blk, :], C4[:, 0, :],
                                    C4[:, 1, :], op=MIN)
                    S2 = spool.tile([P, 8, 512], f16, tag="S2")
                    V.tensor_tensor(S2[:], S1[:, :, 0:512],
                                    S1[:, :, 512:1024], op=MIN)
                    V.tensor_tensor(SS[:, blk, :, :], S2[:, :, 0:256],
                                    S2[:, :, 256:512], op=MIN)
                # Half finale.  COL: combine per-block results, then
                # XBAR-transpose A so column mins become a free-axis
                # fold+reduce (DMA engines move it while VectorE runs
                # the ROW tail).
                D1 = spool.tile([P, 2, _CHUNK], f16, tag="D1")
                V.tensor_tensor(D1[:], CC[:, 0:2, :], CC[:, 2:4, :],
                                op=MIN)
                V.tensor_tensor(A[:], D1[:, 0, :], D1[:, 1, :], op=MIN)
                TA = tpool.tile([P, NB, P], f16, tag="TA", name=f"TA{h}")
                nc.sync.dma_start_transpose(TA[:], A[:])
                T4 = spool.tile([P, NBLK, 8, 128], f16, tag="T4")
                V.tensor_tensor(T4[:], SS[:, :, :, 0:128],
                                SS[:, :, :, 128:256], op=MIN)
                T5 = spool.tile([P, NBLK, 8, 64], f16, tag="T5")
                V.tensor_tensor(T5[:], T4[:, :, :, 0:64],
                                T4[:, :, :, 64:128], op=MIN)
                T6 = spool.tile([P, NBLK, 8, 32], f16, tag="T6")
                V.tensor_tensor(T6[:], T5[:, :, :, 0:32],
                                T5[:, :, :, 32:64], op=MIN)
                Rm = rpool.tile([P, MT], f32, tag="Rm", name=f"Rm{h}")
                V.tensor_reduce(Rm[:], T6[:], axis=AX, op=MIN)
                F1 = spool.tile([P, NB, 64], f16, tag="F1")
                V.tensor_tensor(F1[:], TA[:, :, 0:64], TA[:, :, 64:128],
                                op=MIN)
                F2 = spool.tile([P, NB, 32], f16, tag="F2")
                V.tensor_tensor(F2[:], F1[:, :, 0:32], F1[:, :, 32:64],
                                op=MIN)
                C = rpool.tile([P, NB], f16, tag="C", name=f"C{h}")
                V.tensor_reduce(C[:], F2[:], axis=AX, op=MIN)
                nc.sync.dma_start(
                    dram_out_r[:, h * MT:(h + 1) * MT], Rm[:])
                nc.scalar.dma_start(
                    dram_out_c[:, h * NB:(h + 1) * NB], C[:])

    nc.compile()
    return nc


def _get_program(n_pts=_N):
    if n_pts not in _prog_cache:
        _prog_cache[n_pts] = _build_program(n_pts)
    return _prog_cache[n_pts]


def _split_bf16(x32):
    """x32 fp32 -> (hi, lo) bf16 with hi+lo ~= x to ~2^-18 rel."""
    hi = x32.astype(_BF16)
    lo = (x32 - hi.astype(np.float32)).astype(_BF16)
    return hi, lo


def _split3(x64):
    """fp64 vector -> 3 bf16 terms summing to x to ~2^-27 rel."""
    t0 = x64.astype(_BF16)
    r = x64 - t0.astype(np.float64)
    t1 = r.astype(_BF16)
    r2 = r - t1.astype(np.float64)
    t2 = r2.astype(_BF16)
    return t0, t1, t2


def _features(q, c, n_pts):
    """Feature tensors for the distance matmul.

    q: query points  [3, N] fp32; c: candidate points [3, N] fp32.
    Returns (W [18, N] bf16, R [18, N] bf16) with
      (W.T @ R)[i, j] ~= |q~_i - c~_j|^2
    with ~ the bf16-split (hi+lo) values, exact to ~2e-6.
    """
    q_hi, q_lo = _split_bf16(q)
    c_hi, c_lo = _split_bf16(c)
    q_t = q_hi.astype(np.float32) + q_lo.astype(np.float32)
    c_t = c_hi.astype(np.float32) + c_lo.astype(np.float32)

    U = (c_t.astype(np.float64) ** 2).sum(axis=0)   # candidate norms
    u0, u1, u2 = _split3(U)
    V = (q_t.astype(np.float64) ** 2).sum(axis=0)   # query norms
    v0, v1, v2 = _split3(V)

    m2q_hi = (-2.0 * q_hi.astype(np.float32)).astype(_BF16)
    m2q_lo = (-2.0 * q_lo.astype(np.float32)).astype(_BF16)
    ones = np.ones(n_pts, dtype=_BF16)

    Wg = np.concatenate([
        m2q_hi, m2q_hi, m2q_lo, m2q_lo,
        np.stack([ones, ones, ones]),
        np.stack([v0, v1, v2]),
    ], axis=0).astype(_BF16)              # [18, N]
    Rg = np.concatenate([
        c_hi, c_lo, c_hi, c_lo,
        np.stack([u0, u1, u2]),
        np.stack([ones, ones, ones]),
    ], axis=0).astype(_BF16)              # [18, N]

    return Wg, Rg


def kernel(srcs, tgts):
    import concourse.bass_utils as bass_utils

    srcs = np.asarray(srcs, dtype=np.float32)
    tgts = np.asarray(tgts, dtype=np.float32)
    B = srcs.shape[0]
    assert srcs.shape == (B, _D, _N) and tgts.shape == (B, _D, _N)

    nc = _get_program()

    in_maps = []
    for b in range(B):
        W, R = _features(srcs[b], tgts[b], _N)  # queries = sources
        in_maps.append({"w": W, "r": R})

    res = None
    for attempt in range(3):
        try:
            res = bass_utils.run_bass_kernel_spmd(
                nc, in_maps, core_ids=list(range(_NCORES)),
                trace=TRACE, trace_cores=TRACE_CORES if TRACE else None,
            )
            break
        except Exception:
            # transient NRT/device hiccups have been observed; retry
            if attempt == 2:
                raise
            import time
            time.sleep(3.0)
    global LAST_RESULTS
    LAST_RESULTS = res

    total = 0.0
    for b in range(B):
        out_r = res.results[b]["out_r"]   # [128, 64] f32 per-source mins
        out_c = res.results[b]["out_c"]   # [128, 32] f16 per-target mins
        row = np.minimum(out_r[:, :32], out_r[:, 32:]).astype(np.float64)
        col = out_c.astype(np.float64)
        # reference: min(P, axis=1).mean() -> per-target mins (col);
        #            min(P, axis=2).mean() -> per-source mins (row)
        total += col.mean() + row.mean()

    return np.float32(total / B)
